# revision 44
# baseline (speedup 1.0000x reference)
"""Single-head causal self-attention (B=8, T=2048, D=512, H=64), data-parallel
over batch across 8 NeuronCores. Self-contained: builds a Bass/Tile kernel and
runs it via run_bass_kernel_spmd.

Per-core layout (batch element b = core id), bf16 datapath:
  - the host packs [x ; Wk^T ; Wq^T*s ; Wv^T ; bq*s] into one bf16 DRAM
    tensor xw [2256, 512]; XBAR DMA-transposes deliver both xT (d on
    partitions) and the weight block w_sb in matmul-ready layout -- no PE
    transposes, no separate weight DMAs (mixed DMACopy/transpose prologues
    get serialized by the tile scheduler's cross-DMA chaining)
  - k and q are projected in ONE matmul chain per 512-wide t-chunk
    (lhsT = [Wk | Wq*s] packed [128, 128]); the k bias is dropped (softmax
    over j is invariant to per-row shifts) and bq is added by the
    PSUM->SBUF copy (kT/qT split tiles, cross-partition-base DVE copies)
  - v is projected directly in row layout ([t, h], ap=64 matmuls) into
    v_aug [128, 16, 65] whose last column is ones so the PV matmul also
    accumulates the softmax denominator
  - attention per 512-wide i-block in S^T layout: S^T = k_tile^T @ q, exp
    on ACT (pairs of j-tiles), triangular boundary masks via affine_select
    on Pool; the upper two diagonal tiles are computed on [256:512] only
  - PV uses out[i, h] = e2_tile^T @ v_aug (ap=65), accumulating all four
    128-wide i-subtiles of a block in a single PSUM bank -> output lands
    in row layout, no output transposes
  - dummy PE matmuls at t=0 ramp the tensor engine to full clock while the
    x DMAs are in flight; exp table load is also hoisted to t=0
  - epilogue: reciprocal of the denominator column + per-subtile scale on
    DVE; bv is added on the host (softmax rows sum to 1)
"""

import sys

for _p in ("/root/.axon_site/_ro/trn_rl_repo", "/opt/trn_rl_repo"):
    if _p not in sys.path:
        sys.path.append(_p)

import numpy as np
import ml_dtypes
import concourse.bass as bass
import concourse.bacc as bacc
import concourse.tile as tile
from concourse import mybir
from concourse.bass_utils import run_bass_kernel_spmd

F32 = mybir.dt.float32
BF16 = mybir.dt.bfloat16
EXP = mybir.ActivationFunctionType.Exp

B, T, D, H = 8, 2048, 512, 64
ND = D // 128   # 4 d-chunks
NT = T // 128   # 16 j-tiles
NIB = T // 512  # 4 i-blocks
WROWS = 208     # 192 weight columns + bias row + pad to multiple of 16
N_WARM = 6      # dummy matmuls covering the ~3us PE p-state ramp


def build_body(nc, tc, ctx, dram, repeat=1):
    x_d, out_d = dram

    persist = ctx.enter_context(tc.tile_pool(name="persist", bufs=1))
    epool = ctx.enter_context(tc.tile_pool(name="epool", bufs=4))
    opool = ctx.enter_context(tc.tile_pool(name="opool", bufs=2))
    psKQ = ctx.enter_context(tc.tile_pool(name="psKQ", bufs=2, space="PSUM"))
    psV = ctx.enter_context(tc.tile_pool(name="psV", bufs=1, space="PSUM"))
    psS = ctx.enter_context(tc.tile_pool(name="psS", bufs=2, space="PSUM"))
    psO = ctx.enter_context(tc.tile_pool(name="psO", bufs=1, space="PSUM"))

    wkq_sb = persist.tile([128, ND, 128], BF16)  # [Wk | Wq*s] lhsT per d-chunk
    wv_sb = persist.tile([128, ND, 80], BF16)    # Wv lhsT (+ bq col 64)
    xT = persist.tile([128, 8, ND, 256], BF16)  # [d%128][t256][d//128][t%256]
    kT = persist.tile([64, T], BF16)
    qT = persist.tile([64, T], BF16)
    v_aug = persist.tile([128, NT, 65], BF16)  # v rows + ones column
    warm_mm = persist.tile([128, 512], BF16)
    warm_out = persist.tile([128, 1], BF16)

    # hoist the exp table load to t~0 and ramp the PE to full clock with
    # dummy matmuls while the input DMAs are in flight
    nc.vector.memset(warm_mm[:], 0.0)
    nc.scalar.activation(warm_out[:], warm_mm[:, 0:1], EXP)
    for _ in range(N_WARM):
        wps = psKQ.tile([128, 512], F32, tag="kq")
        nc.tensor.matmul(wps[:], warm_mm[:, 0:128], warm_mm[:],
                         start=True, stop=True)

    # bq*scale arrives in bf16 via the weight transpose (partitions 64:128);
    # tensor_scalar needs a float32 scalar, so widen it once
    bqv = persist.tile([64, 1], F32)
    bq_ap = bqv[:]

    def proj_kq(tch):
        # kq in two 256-wide halves (separate PSUM banks) so the qT/kT
        # PSUM->SBUF copies of half a overlap half b's matmuls; qT first --
        # it gates the next i-block's S matmuls
        for hf in range(2):
            hsl = slice(512 * tch + 256 * hf, 512 * tch + 256 * (hf + 1))
            kq = psKQ.tile([128, 256], F32, tag="kq")
            for dc in range(ND):
                nc.tensor.matmul(kq[:], wkq_sb[:, dc, :],
                                 xT[:, 2 * tch + hf, dc, :],
                                 start=(dc == 0), stop=(dc == ND - 1))
            nc.vector.tensor_scalar_add(qT[:, hsl], kq[64:128, :], bq_ap)
            nc.vector.tensor_copy(kT[:, hsl], kq[0:64, :])

    def proj_v(tch):
        vp = psV.tile([128, 4, 64], F32, tag="v")
        for j in range(4):
            for dc in range(ND):
                nc.tensor.matmul(vp[:, j, :],
                                 xT[:, 2 * tch + j // 2, dc,
                                    128 * (j % 2):128 * (j % 2 + 1)],
                                 wv_sb[:, dc, 0:64],
                                 start=(j == 0 and dc == 0),
                                 stop=(j == 3 and dc == ND - 1),
                                 skip_group_check=True)
        nc.vector.tensor_copy(v_aug[:, 4 * tch:4 * tch + 4, 0:64], vp[:])

    def attn(bi, embed=None, embed2=None, diag_first=False, pending=None):
        i0 = 512 * bi
        o_ps = psO.tile([128, 4, 65], F32, tag="o")
        # pair descriptors: (jtA, jtB, col0, width) -- i-columns [col0, col0+w)
        full = [(2 * p, 2 * p + 1, 0, 512) for p in range(2 * bi)]
        diag = [(4 * bi, 4 * bi + 1, 0, 512),        # diagonal c=0,1
                (4 * bi + 2, 4 * bi + 3, 256, 256)]  # diagonal c=2,3
        pairs = diag + full if diag_first else full + diag
        n_pv = sum(1 for (jA, jB, _, _) in pairs for jt in (jA, jB)
                   for s in range(4) if jt - 4 * bi < 0 or s >= jt - 4 * bi)
        pv_idx = 0

        def emit_pv(e2, desc):
            nonlocal pv_idx
            jtA, jtB, col0, width = desc
            for h, jt in enumerate((jtA, jtB)):
                c = jt - 4 * bi
                for s in range(4):
                    if c >= 0 and s < c:
                        continue
                    ls = 128 * s - col0
                    nc.tensor.matmul(o_ps[:, s, :], e2[:, h, ls:ls + 128],
                                     v_aug[:, jt, :],
                                     start=(pv_idx == 0),
                                     stop=(pv_idx == n_pv - 1),
                                     skip_group_check=True)
                    pv_idx += 1

        prev = None
        for pi, desc in enumerate(pairs):
            jtA, jtB, col0, width = desc
            st = psS.tile([128, 2, 512], F32, tag="s")
            for h, jt in enumerate((jtA, jtB)):
                if bi == 0 and pi == 0:
                    # head: fire each 256-wide half as soon as qT lands
                    for hf in range(2):
                        nc.tensor.matmul(
                            st[:, h, 256 * hf:256 * (hf + 1)],
                            kT[:, 128 * jt:128 * (jt + 1)],
                            qT[:, 256 * hf:256 * (hf + 1)],
                            start=True, stop=True, skip_group_check=True)
                else:
                    nc.tensor.matmul(st[:, h, 0:width],
                                     kT[:, 128 * jt:128 * (jt + 1)],
                                     qT[:, i0 + col0:i0 + col0 + width],
                                     start=True, stop=True)
            e2 = epool.tile([128, 2, 512], BF16, tag="e")
            nc.scalar.activation(e2[:, :, 0:width], st[:, :, 0:width], EXP)
            for h, jt in enumerate((jtA, jtB)):
                c = jt - 4 * bi
                if c >= 0:  # zero the strict upper triangle of the diag tile
                    lo = 128 * c - col0
                    nc.gpsimd.affine_select(
                        out=e2[:, h, lo:lo + 128], in_=e2[:, h, lo:lo + 128],
                        compare_op=mybir.AluOpType.is_ge, fill=0.0,
                        base=0, pattern=[[1, 128]], channel_multiplier=-1)
            if pi == 0 and pending is not None:
                pending()  # previous block's deferred last PV + store
            if pi == 0 and embed is not None:
                embed()   # next t-chunk kq projections (gate the next block)
            if pi == min(1, len(pairs) - 1) and embed2 is not None:
                embed2()  # next t-chunk v projections
            if prev is not None:
                emit_pv(*prev)
            prev = (e2, desc)

        def tail(prev=prev):
            emit_pv(*prev)
            # store numerator+denominator raw; the host does the divide
            o_sb = opool.tile([128, 4, 65], BF16, tag="os")
            nc.vector.tensor_copy(o_sb[:], o_ps[:])
            nc.sync.dma_start(
                out_d[i0:i0 + 512, :].rearrange("(a p) h -> p a h", p=128),
                o_sb[:])
        return tail

    for rep in range(repeat):
        nc.vector.memset(v_aug[:, :, 64:65], 1.0)
        # XBAR-transposed loads (out[p, c, r] = in[r, 128c + p]); keeping the
        # prologue pure-transpose on one queue avoids the tile scheduler's
        # cross-DMA serialization chains. Order = consumption order: kq
        # weights, first x slices, v weights (with bq), remaining x.
        if rep == 0:
            nc.sync.dma_start_transpose(wkq_sb[:], x_d[T:T + 128, :])
        nc.sync.dma_start_transpose(xT[:, 0, :, :], x_d[0:256, :])
        nc.sync.dma_start_transpose(xT[:, 1, :, :], x_d[256:512, :])
        if rep == 0:
            nc.sync.dma_start_transpose(wv_sb[:], x_d[T + 128:T + WROWS, :])
            nc.vector.tensor_copy(bqv[:], wv_sb[64:128, 0, 64:65])
        for sl in range(2, 8):
            nc.sync.dma_start_transpose(
                xT[:, sl, :, :], x_d[256 * sl:256 * (sl + 1), :])

        proj_kq(0)
        proj_v(0)
        t0 = attn(0, embed=lambda: proj_kq(1), embed2=lambda: proj_v(1))
        t1 = attn(1, embed=lambda: proj_kq(2), embed2=lambda: proj_v(2),
                  pending=t0)
        t2 = attn(2, embed=lambda: proj_kq(3), embed2=lambda: proj_v(3),
                  pending=t1)
        t3 = attn(3, diag_first=True, pending=t2)
        t3()


def build_nc(repeat=1):
    nc = bacc.Bacc("TRN2", target_bir_lowering=False, debug=False, num_devices=8)
    x_d = nc.dram_tensor("xw", [T + WROWS, D], BF16, kind="ExternalInput")
    out_d = nc.dram_tensor("out", [T, H + 1], BF16, kind="ExternalOutput")

    from contextlib import ExitStack
    with tile.TileContext(nc) as tc:
        with ExitStack() as ctx:
            build_body(nc, tc, ctx, (x_d, out_d), repeat=repeat)
    nc.compile()
    return nc


_NC_CACHE = {}


def _get_nc(repeat=1):
    if repeat not in _NC_CACHE:
        _NC_CACHE[repeat] = build_nc(repeat)
    return _NC_CACHE[repeat]


def make_in_maps(x, Wk, bk, Wq, bq, Wv, bv):
    scale = float(H) ** -0.5
    bf = ml_dtypes.bfloat16
    # weight block rows r, cols d: w_sb[p, dc, r] = WB[r, 128 dc + p]
    #   r 0:64 = Wk^T, 64:128 = (Wq*s)^T, 128:192 = Wv^T,
    #   r 192 cols 64:128 = bq*s (k-bias dropped: softmax shift invariance)
    WB = np.zeros((WROWS, D), np.float32)
    WB[0:64] = Wk.T
    WB[64:128] = (Wq * scale).T
    WB[128:192] = Wv.T
    WB[192, 64:128] = bq * scale
    WB = WB.astype(bf)
    return [{"xw": np.concatenate([np.asarray(x[b]).astype(bf), WB], axis=0)}
            for b in range(B)]


def kernel(x, Wk, bk, Wq, bq, Wv, bv, _repeat=1):
    x = np.asarray(x, dtype=np.float32)
    Wk = np.asarray(Wk, dtype=np.float32)
    bk = np.asarray(bk, dtype=np.float32)
    Wq = np.asarray(Wq, dtype=np.float32)
    bq = np.asarray(bq, dtype=np.float32)
    Wv = np.asarray(Wv, dtype=np.float32)
    bv = np.asarray(bv, dtype=np.float32)

    nc = _get_nc(_repeat)
    in_maps = make_in_maps(x, Wk, bk, Wq, bq, Wv, bv)
    res = run_bass_kernel_spmd(nc, in_maps, core_ids=list(range(B)))
    raw = np.stack([np.asarray(res.results[b]["out"]).astype(np.float32)
                    for b in range(B)], axis=0)
    return raw[..., 0:64] / raw[..., 64:65] + bv[None, None, :]


# revision 54
# speedup vs baseline: 1.0194x; 1.0194x over previous
"""Single-head causal self-attention (B=8, T=2048, D=512, H=64), data-parallel
over batch across 8 NeuronCores. Self-contained: builds a Bass/Tile kernel and
runs it via run_bass_kernel_spmd.

Per-core layout (batch element b = core id), bf16 datapath:
  - the host packs [x ; Wk^T ; Wq^T*s ; Wv^T ; bq*s] into one bf16 DRAM
    tensor xw [2256, 512]; XBAR DMA-transposes deliver both xT (d on
    partitions) and the weight block w_sb in matmul-ready layout -- no PE
    transposes, no separate weight DMAs (mixed DMACopy/transpose prologues
    get serialized by the tile scheduler's cross-DMA chaining)
  - k and q are projected in ONE matmul chain per 512-wide t-chunk
    (lhsT = [Wk | Wq*s] packed [128, 128]); the k bias is dropped (softmax
    over j is invariant to per-row shifts) and bq is added by the
    PSUM->SBUF copy (kT/qT split tiles, cross-partition-base DVE copies)
  - v is projected directly in row layout ([t, h], ap=64 matmuls) into
    v_aug [128, 16, 65] whose last column is ones so the PV matmul also
    accumulates the softmax denominator
  - attention per 512-wide i-block in S^T layout: S^T = k_tile^T @ q, exp
    on ACT (pairs of j-tiles), triangular boundary masks via affine_select
    on Pool; the upper two diagonal tiles are computed on [256:512] only
  - PV uses out[i, h] = e2_tile^T @ v_aug (ap=65), accumulating all four
    128-wide i-subtiles of a block in a single PSUM bank -> output lands
    in row layout, no output transposes
  - dummy PE matmuls at t=0 ramp the tensor engine to full clock while the
    x DMAs are in flight; exp table load is also hoisted to t=0
  - epilogue: reciprocal of the denominator column + per-subtile scale on
    DVE; bv is added on the host (softmax rows sum to 1)
"""

import sys

for _p in ("/root/.axon_site/_ro/trn_rl_repo", "/opt/trn_rl_repo"):
    if _p not in sys.path:
        sys.path.append(_p)

import numpy as np
import ml_dtypes
import concourse.bass as bass
import concourse.bacc as bacc
import concourse.tile as tile
from concourse import mybir
from concourse.bass_utils import run_bass_kernel_spmd

F32 = mybir.dt.float32
BF16 = mybir.dt.bfloat16
EXP = mybir.ActivationFunctionType.Exp

B, T, D, H = 8, 2048, 512, 64
ND = D // 128   # 4 d-chunks
NT = T // 128   # 16 j-tiles
NIB = T // 512  # 4 i-blocks
WROWS = 208     # 192 weight columns + bias row + pad to multiple of 16
N_WARM = 6      # dummy matmuls covering the ~3us PE p-state ramp
import os
TUNE = {k: int(v) for k, v in
        (kv.split("=") for kv in os.environ.get("KTUNE", "").split(",") if kv)}
N_WARM = TUNE.get("warm", N_WARM)
EBUFS_DEFAULT = 6
E2BACK_DEFAULT = 1


def build_body(nc, tc, ctx, dram, repeat=1):
    x_d, out_d = dram

    persist = ctx.enter_context(tc.tile_pool(name="persist", bufs=1))
    epool = ctx.enter_context(tc.tile_pool(name="epool", bufs=TUNE.get("ebufs", 8)))
    opool = ctx.enter_context(tc.tile_pool(name="opool", bufs=2))
    psKQ = ctx.enter_context(tc.tile_pool(name="psKQ", bufs=2, space="PSUM"))
    psV = ctx.enter_context(tc.tile_pool(name="psV", bufs=1, space="PSUM"))
    psS = ctx.enter_context(tc.tile_pool(name="psS", bufs=2, space="PSUM"))
    psO = ctx.enter_context(tc.tile_pool(name="psO", bufs=1, space="PSUM"))

    wkq_sb = persist.tile([128, ND, 128], BF16)  # [Wk | Wq*s] lhsT per d-chunk
    wv_sb = persist.tile([128, ND, 80], BF16)    # Wv lhsT (+ bq col 64)
    xT = persist.tile([128, 8, ND, 256], BF16)  # [d%128][t256][d//128][t%256]
    kT = persist.tile([64, T], BF16)
    qT = persist.tile([64, T], BF16)
    v_aug = persist.tile([128, NT, 65], BF16)  # v rows + ones column
    warm_mm = persist.tile([128, 512], BF16)
    warm_out = persist.tile([128, 1], BF16)

    # hoist the exp table load to t~0 and ramp the PE to full clock with
    # dummy matmuls while the input DMAs are in flight
    nc.vector.memset(warm_mm[:], 0.0)
    nc.scalar.activation(warm_out[:], warm_mm[:, 0:1], EXP)
    for _ in range(N_WARM):
        wps = psKQ.tile([128, 512], F32, tag="kq")
        nc.tensor.matmul(wps[:], warm_mm[:, 0:128], warm_mm[:],
                         start=True, stop=True)

    # bq*scale arrives in bf16 via the weight transpose (partitions 64:128);
    # tensor_scalar needs a float32 scalar, so widen it once
    bqv = persist.tile([64, 1], F32)
    bq_ap = bqv[:]

    def proj_kq(tch, act_kt=False):
        # kq in two 256-wide halves (separate PSUM banks) so the qT/kT
        # PSUM->SBUF copies of half a overlap half b's matmuls; qT first --
        # it gates the next i-block's S matmuls
        for hf in range(2):
            hsl = slice(512 * tch + 256 * hf, 512 * tch + 256 * (hf + 1))
            kq = psKQ.tile([128, 256], F32, tag="kq")
            for dc in range(ND):
                nc.tensor.matmul(kq[:], wkq_sb[:, dc, :],
                                 xT[:, 2 * tch + hf, dc, :],
                                 start=(dc == 0), stop=(dc == ND - 1))
            nc.vector.tensor_scalar_add(qT[:, hsl], kq[64:128, :], bq_ap)
            if act_kt:  # head: ACT is idle before the first exp
                nc.scalar.activation(kT[:, hsl], kq[0:64, :],
                                     mybir.ActivationFunctionType.Copy)
            else:
                nc.vector.tensor_copy(kT[:, hsl], kq[0:64, :])

    def proj_v(tch):
        vp = psV.tile([128, 4, 64], F32, tag="v")
        for j in range(4):
            for dc in range(ND):
                nc.tensor.matmul(vp[:, j, :],
                                 xT[:, 2 * tch + j // 2, dc,
                                    128 * (j % 2):128 * (j % 2 + 1)],
                                 wv_sb[:, dc, 0:64],
                                 start=(j == 0 and dc == 0),
                                 stop=(j == 3 and dc == ND - 1),
                                 skip_group_check=True)
        nc.vector.tensor_copy(v_aug[:, 4 * tch:4 * tch + 4, 0:64], vp[:])

    def attn(bi, embed=None, embed2=None, diag_first=False, pending=None):
        i0 = 512 * bi
        o_ps = psO.tile([128, 4, 65], F32, tag="o")
        # pair descriptors: (jtA, jtB, col0, width) -- i-columns [col0, col0+w)
        full = [(2 * p, 2 * p + 1, 0, 512) for p in range(2 * bi)]
        diag = [(4 * bi, 4 * bi + 1, 0, 512),        # diagonal c=0,1
                (4 * bi + 2, 4 * bi + 3, 256, 256)]  # diagonal c=2,3
        pairs = diag + full if diag_first else full + diag
        n_pv = sum(1 for (jA, jB, _, _) in pairs for jt in (jA, jB)
                   for s in range(4) if jt - 4 * bi < 0 or s >= jt - 4 * bi)
        pv_idx = 0

        def emit_pv(e2, desc, s_major=False):
            nonlocal pv_idx
            jtA, jtB, col0, width = desc
            order = [(h, jt, s) for s in range(4)
                     for h, jt in enumerate((jtA, jtB))] if s_major else \
                    [(h, jt, s) for h, jt in enumerate((jtA, jtB))
                     for s in range(4)]
            for h, jt, s in order:
                c = jt - 4 * bi
                if c >= 0 and s < c:
                    continue
                ls = 128 * s - col0
                nc.tensor.matmul(o_ps[:, s, :], e2[:, h, ls:ls + 128],
                                 v_aug[:, jt, :],
                                 start=(pv_idx == 0),
                                 stop=(pv_idx == n_pv - 1),
                                 skip_group_check=True)
                pv_idx += 1

        prev = None
        for pi, desc in enumerate(pairs):
            jtA, jtB, col0, width = desc
            st = psS.tile([128, 2, 512], F32, tag="s")
            for h, jt in enumerate((jtA, jtB)):
                c = jt - 4 * bi
                # diagonal tiles only need columns i >= 128c; narrower S
                # matmuls leave stale-but-finite PSUM in the dead region,
                # which exp may read but PV never consumes
                lo = max(col0, 128 * c) if c > 0 else col0
                if bi == 0 and pi == 0:
                    # head: fire each 256-wide half as soon as qT lands
                    for hf in range(2):
                        if 256 * (hf + 1) <= lo:
                            continue
                        l0 = max(lo, 256 * hf)
                        nc.tensor.matmul(
                            st[:, h, l0:256 * (hf + 1)],
                            kT[:, 128 * jt:128 * (jt + 1)],
                            qT[:, l0:256 * (hf + 1)],
                            start=True, stop=True, skip_group_check=True)
                else:
                    nc.tensor.matmul(st[:, h, lo - col0:width],
                                     kT[:, 128 * jt:128 * (jt + 1)],
                                     qT[:, i0 + lo:i0 + col0 + width],
                                     start=True, stop=True)
            e2 = epool.tile([128, 2, 512], BF16, tag="e")
            if bi == 0 and pi == 0:
                # head: exp each i-half as its S matmuls complete
                nc.scalar.activation(e2[:, :, 0:256], st[:, :, 0:256], EXP)
                nc.scalar.activation(e2[:, :, 256:512], st[:, :, 256:512], EXP)
            else:
                nc.scalar.activation(e2[:, :, 0:width], st[:, :, 0:width], EXP)
            for h, jt in enumerate((jtA, jtB)):
                c = jt - 4 * bi
                if c >= 0:  # zero the strict upper triangle of the diag tile
                    lo = 128 * c - col0
                    nc.gpsimd.affine_select(
                        out=e2[:, h, lo:lo + 128], in_=e2[:, h, lo:lo + 128],
                        compare_op=mybir.AluOpType.is_ge, fill=0.0,
                        base=0, pattern=[[1, 128]], channel_multiplier=-1)
            if pi == 0 and pending is not None:
                pending()  # previous block's deferred last PV + store
            if pi == min(TUNE.get("e1", 0), len(pairs) - 1) and embed is not None:
                embed()   # next t-chunk kq projections (gate the next block)
            e2at = len(pairs) - 1 - TUNE.get("e2back", E2BACK_DEFAULT)
            if pi == max(0, e2at) and embed2 is not None:
                embed2()  # next t-chunk v projections
            if prev is not None:
                emit_pv(*prev)
            prev = (e2, desc)

        def tail(prev=prev, last=False):
            emit_pv(*prev, s_major=last)
            # store numerator+denominator raw; the host does the divide
            o_sb = opool.tile([128, 4, 65], BF16, tag="os")
            if last:  # drain the early subtiles while the rest finish
                for half in range(2):
                    nc.vector.tensor_copy(o_sb[:, 2 * half:2 * half + 2, :],
                                          o_ps[:, 2 * half:2 * half + 2, :])
                    nc.sync.dma_start(
                        out_d[i0 + 256 * half:i0 + 256 * (half + 1), :]
                        .rearrange("(a p) h -> p a h", p=128),
                        o_sb[:, 2 * half:2 * half + 2, :])
            else:
                nc.vector.tensor_copy(o_sb[:], o_ps[:])
                nc.sync.dma_start(
                    out_d[i0:i0 + 512, :].rearrange("(a p) h -> p a h", p=128),
                    o_sb[:])
        return tail

    for rep in range(repeat):
        nc.vector.memset(v_aug[:, :, 64:65], 1.0)
        # XBAR-transposed loads (out[p, c, r] = in[r, 128c + p]); keeping the
        # prologue pure-transpose on one queue avoids the tile scheduler's
        # cross-DMA serialization chains. Order = consumption order: kq
        # weights, first x slices, v weights (with bq), remaining x.
        nc.sync.dma_start_transpose(xT[:, 0, :, :], x_d[0:256, :])
        if rep == 0:
            nc.sync.dma_start_transpose(wkq_sb[:], x_d[T:T + 128, :])
        nc.sync.dma_start_transpose(xT[:, 1, :, :], x_d[256:512, :])
        if rep == 0:
            nc.sync.dma_start_transpose(wv_sb[:], x_d[T + 128:T + WROWS, :])
            nc.vector.tensor_copy(bqv[:], wv_sb[64:128, 0, 64:65])
        for sl in range(2, 8):
            nc.sync.dma_start_transpose(
                xT[:, sl, :, :], x_d[256 * sl:256 * (sl + 1), :])

        proj_kq(0, act_kt=True)
        proj_v(0)
        t0 = attn(0, embed=lambda: proj_kq(1), embed2=lambda: proj_v(1))
        t1 = attn(1, embed=lambda: proj_kq(2), embed2=lambda: proj_v(2),
                  pending=t0)
        t2 = attn(2, embed=lambda: proj_kq(3), embed2=lambda: proj_v(3),
                  pending=t1)
        t3 = attn(3, diag_first=True, pending=t2)
        t3()


def build_nc(repeat=1):
    nc = bacc.Bacc("TRN2", target_bir_lowering=False, debug=False, num_devices=8)
    x_d = nc.dram_tensor("xw", [T + WROWS, D], BF16, kind="ExternalInput")
    out_d = nc.dram_tensor("out", [T, H + 1], BF16, kind="ExternalOutput")

    from contextlib import ExitStack
    with tile.TileContext(nc) as tc:
        with ExitStack() as ctx:
            build_body(nc, tc, ctx, (x_d, out_d), repeat=repeat)
    nc.compile()
    return nc


_NC_CACHE = {}


def _get_nc(repeat=1):
    if repeat not in _NC_CACHE:
        _NC_CACHE[repeat] = build_nc(repeat)
    return _NC_CACHE[repeat]


def make_in_maps(x, Wk, bk, Wq, bq, Wv, bv):
    scale = float(H) ** -0.5
    bf = ml_dtypes.bfloat16
    # weight block rows r, cols d: w_sb[p, dc, r] = WB[r, 128 dc + p]
    #   r 0:64 = Wk^T, 64:128 = (Wq*s)^T, 128:192 = Wv^T,
    #   r 192 cols 64:128 = bq*s (k-bias dropped: softmax shift invariance)
    WB = np.zeros((WROWS, D), np.float32)
    WB[0:64] = Wk.T
    WB[64:128] = (Wq * scale).T
    WB[128:192] = Wv.T
    WB[192, 64:128] = bq * scale
    WB = WB.astype(bf)
    return [{"xw": np.concatenate([np.asarray(x[b]).astype(bf), WB], axis=0)}
            for b in range(B)]


def kernel(x, Wk, bk, Wq, bq, Wv, bv, _repeat=1):
    x = np.asarray(x, dtype=np.float32)
    Wk = np.asarray(Wk, dtype=np.float32)
    bk = np.asarray(bk, dtype=np.float32)
    Wq = np.asarray(Wq, dtype=np.float32)
    bq = np.asarray(bq, dtype=np.float32)
    Wv = np.asarray(Wv, dtype=np.float32)
    bv = np.asarray(bv, dtype=np.float32)

    nc = _get_nc(_repeat)
    in_maps = make_in_maps(x, Wk, bk, Wq, bq, Wv, bv)
    res = run_bass_kernel_spmd(nc, in_maps, core_ids=list(range(B)))
    raw = np.stack([np.asarray(res.results[b]["out"]).astype(np.float32)
                    for b in range(B)], axis=0)
    return raw[..., 0:64] / raw[..., 64:65] + bv[None, None, :]
